# revision 10
# baseline (speedup 1.0000x reference)
"""Self-contained Trainium2 Bass kernel for the 2-layer GAT
(nn_GAT_18915035971953): 100000 nodes, 1.6M edges, 8 NeuronCores.

Strategy: edges sorted by destination, dst-sharded across 8 cores
(12500 dst nodes each).  Edges live in SBUF partitions; per 128-dst
window a one-hot matrix (built by DVE is_equal) turns segment-sum into
PSUM matmuls.  v2 reorganization vs the original baseline:
  - gather calls merged across window GROUPS (~64 slots each): one
    dma_gather per (group, src-bucket) + one a_d gather per group,
    ~5 calls per group instead of ~13 per window (GpSimd desc-gen was
    the #1 bottleneck at ~1.7ms/launch).
  - layer-1 a_d comes from the same dc-gather as layer 2 (the per-slot
    PE-transpose trick burned 0.9ms of Tensor+Scalar time).
  - feature rows are head-innermost [32f x 8h] so the per-edge softmax
    scale w broadcasts over a middle dim: every DVE operand keeps a
    stride-1 16-bit innermost dim -> 2x_1p mode on the big multiply.
  - w is written over the a_s slots of the gathered tile, making the
    gathered tile itself the matmul rhs (no rhs copy, less SBUF).
Three SPMD launches: dense layer-1 tables -> layer-1 edge phase ->
layer-2 edge phase; the host only reorders indices and concatenates
shard outputs between launches.
"""
import sys
from dataclasses import dataclass
import numpy as np
import ml_dtypes

if "/opt/trn_rl_repo" not in sys.path:
    sys.path.insert(0, "/opt/trn_rl_repo")

import concourse.bacc as bacc
import concourse.mybir as mybir
import concourse.tile as tile
from concourse.masks import make_identity
from concourse import bass_utils

P = 128
H = 8
POISON = -1.0e30
NQ = 4
SGRP = 64                # max slots per window-group (gather-call merge unit)
MAXNIDX = 1024           # max idxs per dma_gather call (HW-validated bound)
F32 = mybir.dt.float32
BF16 = mybir.dt.bfloat16
I16 = mybir.dt.int16
AF = mybir.ActivationFunctionType
ALU = mybir.AluOpType
AX = mybir.AxisListType


@dataclass
class Dims:
    N: int = 100000          # total nodes
    NCORES: int = 8
    NBUCK: int = 4           # src buckets (int16 gather indices < 32768)

    @property
    def NPC(self):
        return self.N // self.NCORES

    @property
    def BUCK(self):
        return self.N // self.NBUCK


def _wrap16(idx):
    n = idx.shape[0]
    assert n % 16 == 0
    w = idx.reshape(n // 16, 16).T.astype(np.int16)
    return np.tile(w, (8, 1))


def build_plan(edge_index, dims: Dims, sgrp=SGRP):
    """Group-major edge plan.

    Slot order: [group0: [bucket0: w0,w1..  bucket1: w0,w1.. ...]
                 group1: ...]
    Returns (plan, streams).
    plan: dict(slots, groups=[{SG, slot0, calls=[(b, n, gc)],
               windows=[{w, r, segs=[(scol, ns)]}]}])
    streams per core: gidx int16 [128, slots*8], dcidx int16 [128, slots*8],
    dstloc bf16 [128, slots].
    Pads: gather idx -> row 0 of bucket (finite data), dc idx -> poison row
    NPC (a_d = -1e30 -> w = 0), dstloc -> -1 (one-hot all-zero).
    """
    N, NC, NB, BUCK = dims.N, dims.NCORES, dims.NBUCK, dims.BUCK
    NPC = dims.NPC
    src = np.asarray(edge_index[0], np.int64)
    dst = np.asarray(edge_index[1], np.int64)
    order = np.argsort(dst, kind="stable")
    s_src, s_dst = src[order], dst[order]
    counts = np.bincount(s_dst, minlength=N)
    node_start = np.concatenate([[0], np.cumsum(counts)])

    nwin = (NPC + P - 1) // P
    seg = [[[None] * NB for _ in range(nwin)] for _ in range(NC)]
    for c in range(NC):
        d0 = c * NPC
        for w in range(nwin):
            lo = node_start[d0 + w * P]
            hi = node_start[min(d0 + (w + 1) * P, d0 + NPC)]
            esrc = s_src[lo:hi]
            edst = s_dst[lo:hi]
            for b in range(NB):
                m = (esrc // BUCK) == b
                seg[c][w][b] = (esrc[m], edst[m])

    # slots per (window, bucket): max over cores (SPMD shares the program)
    S_wb = np.zeros((nwin, NB), np.int64)
    for w in range(nwin):
        for b in range(NB):
            kmax = max(seg[c][w][b][0].shape[0] for c in range(NC))
            S_wb[w, b] = (kmax + P - 1) // P
    S_w = S_wb.sum(axis=1)

    # greedy window groups
    grp_lists = []
    cur, tot = [], 0
    for w in range(nwin):
        if cur and tot + S_w[w] > sgrp:
            grp_lists.append(cur)
            cur, tot = [], 0
        cur.append(w)
        tot += S_w[w]
    if cur:
        grp_lists.append(cur)

    groups = []
    slot0 = 0
    core_g = [[] for _ in range(NC)]
    core_dc = [[] for _ in range(NC)]
    core_dl = [[] for _ in range(NC)]
    for wl in grp_lists:
        calls = []
        segs = {w: [] for w in wl}
        scol = 0
        gc = 0
        for b in range(NB):
            n_b = 0
            for w in wl:
                ns = int(S_wb[w, b])
                if ns == 0:
                    continue
                segs[w].append((scol, ns))
                nn = ns * P
                n_b += nn
                scol += ns
                d0base = [c * NPC for c in range(NC)]
                for c in range(NC):
                    es, ed = seg[c][w][b]
                    k = es.shape[0]
                    gi = np.concatenate(
                        [es - b * BUCK, np.zeros(nn - k, np.int64)])
                    dc = np.concatenate(
                        [ed - d0base[c], np.full(nn - k, NPC, np.int64)])
                    dl = np.concatenate(
                        [ed - (d0base[c] + w * P), np.full(nn - k, -1, np.int64)])
                    core_g[c].append(_wrap16(gi))
                    core_dc[c].append(_wrap16(dc))
                    core_dl[c].append(dl.reshape(nn // P, P).T)
            if n_b:
                calls.append((b, n_b, gc))
                gc += n_b // 16
        if scol == 0:
            # degenerate empty group: one pad slot
            calls.append((0, P, 0))
            for w in wl:
                pass
            for c in range(NC):
                core_g[c].append(_wrap16(np.zeros(P, np.int64)))
                core_dc[c].append(_wrap16(np.full(P, NPC, np.int64)))
                core_dl[c].append(np.full((P, 1), -1, np.int64))
            scol = 1
        windows = []
        for w in wl:
            r = min(P, NPC - w * P)
            windows.append(dict(w=w, r=r, segs=segs[w]))
        groups.append(dict(SG=scol, slot0=slot0, calls=calls, windows=windows))
        slot0 += scol

    plan = dict(slots=slot0, nwin=nwin, groups=groups)
    streams = []
    for c in range(NC):
        streams.append(dict(
            gidx=np.ascontiguousarray(np.concatenate(core_g[c], axis=1)),
            dcidx=np.ascontiguousarray(np.concatenate(core_dc[c], axis=1)),
            dstloc=np.ascontiguousarray(
                np.concatenate(core_dl[c], axis=1).astype(ml_dtypes.bfloat16)),
        ))
    return plan, streams


# ---------------- kernel builders ----------------


def build_dense1(dims):
    """h1 = x @ W1perm (head-innermost cols); writes T1 [NPC,384] bf16
    rows = [256 feats | a_s as f32 | pad] and AD1 [NPC+1, 64] f32."""
    NPC = dims.NPC
    nc = bacc.Bacc(None, target_bir_lowering=False, num_swdge_queues=NQ)
    with tile.TileContext(nc) as tc:
        with tc.tile_pool(name="dram", bufs=1, space="DRAM") as dram:
            xT = dram.tile([P, NPC], BF16, kind="ExternalInput")
            W1 = dram.tile([P, 256], F32, kind="ExternalInput")
            att1 = dram.tile([1, 512], F32, kind="ExternalInput")
            T1 = dram.tile([NPC, 384], BF16, kind="ExternalOutput")
            AD1 = dram.tile([NPC + 1, 64], F32, kind="ExternalOutput")
            names = dict(xT=xT.name, W1=W1.name, att1=att1.name,
                         T1=T1.name, AD1=AD1.name)
            with tc.tile_pool(name="cst", bufs=1) as cst, \
                 tc.tile_pool(name="wk", bufs=3) as wk, \
                 tc.tile_pool(name="ps", bufs=4, space="PSUM") as ps:
                xTs = cst.tile([P, NPC], BF16)
                nc.sync.dma_start(xTs[:], xT[:])
                rhs = cst.tile([P, 272], F32)
                nc.sync.dma_start(rhs[:, 0:256], W1[:])
                att_s = cst.tile([1, 512], F32)
                nc.sync.dma_start(att_s[:], att1[:])
                attb = cst.tile([P, 512], F32)
                nc.gpsimd.partition_broadcast(attb[:, 0:256], att_s[0:1, 0:256])
                nc.gpsimd.partition_broadcast(attb[:, 256:512], att_s[0:1, 256:512])
                tmp = cst.tile([P, 512], F32)
                nc.vector.tensor_tensor(out=tmp[:, 0:256], in0=rhs[:, 0:256],
                                        in1=attb[:, 0:256], op=ALU.mult)
                nc.vector.tensor_tensor(out=tmp[:, 256:512], in0=rhs[:, 0:256],
                                        in1=attb[:, 256:512], op=ALU.mult)
                # cols are (f*8+h); a_s[h] = sum_f -> strided view [h, f]
                tv = tmp[:].rearrange("p (v f h) -> p v h f", v=2, h=H)
                nc.vector.tensor_reduce(out=rhs[:, 256:264], in_=tv[:, 0],
                                        axis=AX.X, op=ALU.add)
                nc.vector.tensor_reduce(out=rhs[:, 264:272], in_=tv[:, 1],
                                        axis=AX.X, op=ALU.add)
                rhs_bf = cst.tile([P, 272], BF16)
                nc.vector.tensor_copy(rhs_bf[:], rhs[:])
                ntile = (NPC + P - 1) // P
                for i in range(ntile):
                    r = min(P, NPC - i * P)
                    po = ps.tile([P, 272], F32, tag="po")
                    nc.tensor.matmul(out=po[:r, :], lhsT=xTs[:, i * P:i * P + r],
                                     rhs=rhs_bf[:], start=True, stop=True)
                    t1t = wk.tile([P, 384], BF16, tag="t1t")
                    nc.vector.memset(t1t[:], 0.0)
                    nc.scalar.copy(t1t[:r, 0:256], po[:r, 0:256])
                    nc.vector.tensor_copy(t1t[:, 256:384].bitcast(F32)[:r, 0:8],
                                          po[:r, 256:264])
                    nc.sync.dma_start(T1[i * P:i * P + r, :], t1t[:r, :])
                    adt = wk.tile([P, 64], F32, tag="adt")
                    nc.vector.memset(adt[:], 0.0)
                    nc.vector.tensor_copy(adt[:r, 0:8], po[:r, 264:272])
                    nc.sync.dma_start(AD1[i * P:i * P + r, :], adt[:r, :])
                pois = wk.tile([1, 64], F32, tag="pois")
                nc.vector.memset(pois[:], POISON)
                nc.sync.dma_start(AD1[NPC:NPC + 1, :], pois[:])
    nc.compile()
    return nc, names


def build_edge(layer, plan, dims):
    N, NPC, BUCK = dims.N, dims.NPC, dims.BUCK
    groups = plan['groups']
    slots = plan['slots']
    GW = 384 if layer == 1 else 64          # gather row width (elements)
    GDT = BF16 if layer == 1 else F32
    nc = bacc.Bacc(None, target_bir_lowering=False, num_swdge_queues=NQ)
    qctr = [0]

    def nextq():
        q = qctr[0] % NQ
        qctr[0] += 1
        return q

    with tile.TileContext(nc) as tc:
        with tc.tile_pool(name="dram", bufs=1, space="DRAM") as dram:
            names = {}
            Gt = dram.tile([N, GW], GDT, kind="ExternalInput")
            ADt = dram.tile([NPC + 1, 64], F32, kind="ExternalInput")
            W2 = dram.tile([32, 128], F32, kind="ExternalInput")
            nb = 32 if layer == 1 else 16
            bias = dram.tile([1, nb], F32, kind="ExternalInput")
            names.update(G=Gt.name, AD=ADt.name, W2=W2.name, bias=bias.name)
            if layer == 1:
                att2 = dram.tile([1, 256], F32, kind="ExternalInput")
                T2o = dram.tile([NPC, 64], F32, kind="ExternalOutput")
                AD2o = dram.tile([NPC + 1, 64], F32, kind="ExternalOutput")
                names.update(att2=att2.name, T2=T2o.name, AD2=AD2o.name)
            else:
                OUT = dram.tile([NPC, 16], F32, kind="ExternalOutput")
                names.update(OUT=OUT.name)
            gidx = dram.tile([P, slots * 8], I16, kind="ExternalInput")
            dcidx = dram.tile([P, slots * 8], I16, kind="ExternalInput")
            dstloc = dram.tile([P, slots], BF16, kind="ExternalInput")
            names.update(gidx=gidx.name, dcidx=dcidx.name, dstloc=dstloc.name)

            with tc.tile_pool(name="cst", bufs=1) as cst, \
                 tc.tile_pool(name="ix", bufs=2) as ix, \
                 tc.tile_pool(name="gp", bufs=2) as gp, \
                 tc.tile_pool(name="dp", bufs=2) as dp, \
                 tc.tile_pool(name="pp", bufs=2) as pp, \
                 tc.tile_pool(name="wk", bufs=2) as wk, \
                 tc.tile_pool(name="psa", bufs=4, space="PSUM") as psa, \
                 tc.tile_pool(name="pst", bufs=2, space="PSUM") as pst, \
                 tc.tile_pool(name="pso", bufs=2, space="PSUM") as pso:
                iota_i = cst.tile([P, P], mybir.dt.int32)
                nc.gpsimd.iota(iota_i[:], pattern=[[1, P]], base=0,
                               channel_multiplier=0)
                iota_bf = cst.tile([P, P], BF16)
                nc.vector.tensor_copy(iota_bf[:], iota_i[:])
                bias_s = cst.tile([1, nb], F32)
                nc.sync.dma_start(bias_s[:], bias[:])
                bias_b = cst.tile([P, nb], F32)
                nc.gpsimd.partition_broadcast(bias_b[:], bias_s[0:1, :])
                W2s = cst.tile([32, 128], F32)
                nc.sync.dma_start(W2s[:], W2[:])
                ident_bf = cst.tile([P, P], BF16)
                make_identity(nc, ident_bf[:])
                if layer == 1:
                    att2_s = cst.tile([1, 256], F32)
                    nc.sync.dma_start(att2_s[:], att2[:])
                    att2b = cst.tile([32, 256], F32)
                    nc.gpsimd.partition_broadcast(att2b[:, 0:128], att2_s[0:1, 0:128])
                    nc.gpsimd.partition_broadcast(att2b[:, 128:256], att2_s[0:1, 128:256])
                    tmp2 = cst.tile([32, 256], F32)
                    nc.vector.tensor_tensor(out=tmp2[:, 0:128], in0=W2s[:],
                                            in1=att2b[:, 0:128], op=ALU.mult)
                    nc.vector.tensor_tensor(out=tmp2[:, 128:256], in0=W2s[:],
                                            in1=att2b[:, 128:256], op=ALU.mult)
                    t2v = tmp2[:].rearrange("p (v h f) -> p v h f", v=2, h=H)
                    wt2 = cst.tile([32, 16], F32)
                    nc.vector.tensor_reduce(out=wt2[:, 0:8], in_=t2v[:, 0],
                                            axis=AX.X, op=ALU.add)
                    nc.vector.tensor_reduce(out=wt2[:, 8:16], in_=t2v[:, 1],
                                            axis=AX.X, op=ALU.add)
                    ident = cst.tile([P, P], F32)
                    make_identity(nc, ident[:])
                else:
                    w2r0 = cst.tile([P, 16], BF16)
                    w2r1 = cst.tile([P, 16], BF16)
                    for h in range(H):
                        dsttile = w2r0 if h < 4 else w2r1
                        nc.vector.tensor_copy(
                            dsttile[(h % 4) * 32:(h % 4 + 1) * 32, :],
                            W2s[:, h * 16:(h + 1) * 16])

                for g in groups:
                    SG = g['SG']
                    s0 = g['slot0']
                    ixg = ix.tile([P, SG * 8], I16, tag="ixg")
                    nc.sync.dma_start(ixg[:], gidx[:, s0 * 8:(s0 + SG) * 8])
                    ixd = ix.tile([P, SG * 8], I16, tag="ixd")
                    nc.sync.dma_start(ixd[:], dcidx[:, s0 * 8:(s0 + SG) * 8])
                    dl = ix.tile([P, SG], BF16, tag="dl")
                    nc.sync.dma_start(dl[:], dstloc[:, s0:s0 + SG])

                    g_t = gp.tile([P, SG, GW], GDT, tag="g")
                    for (b, n, gc) in g['calls']:
                        off = 0
                        while off < n:
                            take = min(MAXNIDX, n - off)
                            gc2 = gc + off // 16
                            scol = gc2 * 16 // P
                            nc.gpsimd.dma_gather(
                                g_t[:, scol:scol + take // P, :],
                                Gt[b * BUCK:(b + 1) * BUCK, :],
                                ixg[:, gc2:gc2 + take // 16], take, take, GW,
                                queue_num=nextq())
                            off += take
                    dc_t = dp.tile([P, SG, 64], F32, tag="dc")
                    off = 0
                    while off < SG * P:
                        take = min(MAXNIDX, SG * P - off)
                        scol = off // P
                        nc.gpsimd.dma_gather(
                            dc_t[:, scol:scol + take // P, :], ADt[:],
                            ixd[:, off // 16:(off + take) // 16], take, take, 64,
                            queue_num=nextq())
                        off += take

                    p_t = pp.tile([P, SG, P], BF16, tag="pt")
                    dl_b = dl[:].unsqueeze(2).to_broadcast([P, SG, P])
                    io_b = iota_bf[:].unsqueeze(1).to_broadcast([P, SG, P])
                    nc.vector.tensor_tensor(out=p_t[:], in0=dl_b, in1=io_b,
                                            op=ALU.is_equal)

                    et = wk.tile([P, SG, 8], F32, tag="et")
                    if layer == 1:
                        a_s_ap = g_t[:].rearrange("p s e -> p (s e)") \
                            .bitcast(F32).rearrange("p (s e) -> p s e", e=192)[:, :, 128:136]
                    else:
                        a_s_ap = g_t[:, :, 32:40]
                    nc.vector.tensor_tensor(out=et[:], in0=a_s_ap,
                                            in1=dc_t[:, :, 0:8], op=ALU.add)
                    nc.vector.scalar_tensor_tensor(
                        out=et[:], in0=et[:], scalar=0.2, in1=et[:],
                        op0=ALU.mult, op1=ALU.max)
                    if layer == 1:
                        # w -> bf16 cols 256:264 of the gathered tile
                        # (overwrites a_s low halves; a_s fully consumed)
                        nc.scalar.activation(g_t[:, :, 256:264], et[:], AF.Exp)
                        # in-place msg scale, 2x_1p: [P,SG,32f,8h] * w[.,.,1,8]
                        gv = g_t[:, :, 0:256].rearrange(
                            "p s (f h) -> p s f h", h=H)
                        w_b = g_t[:, :, 256:264].unsqueeze(2) \
                            .to_broadcast([P, SG, 32, H])
                        nc.vector.tensor_tensor(out=gv, in0=gv, in1=w_b,
                                                op=ALU.mult)
                        rhs_view = g_t[:, :, 0:264]
                    else:
                        w_t = wk.tile([P, SG, 8], BF16, tag="wt")
                        nc.scalar.activation(w_t[:], et[:], AF.Exp)
                        rhs_t = wk.tile([P, SG, 264], BF16, tag="rhs")
                        x2_b = g_t[:, :, 0:32].unsqueeze(2) \
                            .to_broadcast([P, SG, H, 32])
                        wv_b = w_t[:].unsqueeze(3).to_broadcast([P, SG, H, 32])
                        nc.vector.tensor_tensor(
                            out=rhs_t[:, :, 0:256].rearrange(
                                "p s (h f) -> p s h f", h=H),
                            in0=x2_b, in1=wv_b, op=ALU.mult)
                        nc.vector.tensor_copy(rhs_t[:, :, 256:264], w_t[:])
                        rhs_view = rhs_t[:, :, :]

                    for win in g['windows']:
                        w = win['w']
                        r = win['r']
                        segs = win['segs']
                        total = sum(ns for (_, ns) in segs)
                        if total == 0:
                            continue
                        agg = psa.tile([P, 264], F32, tag="agg")
                        k = 0
                        for (scol, ns) in segs:
                            for s in range(scol, scol + ns):
                                nc.tensor.matmul(
                                    out=agg[:], lhsT=p_t[:, s, :],
                                    rhs=rhs_view[:, s, :],
                                    start=(k == 0), stop=(k == total - 1))
                                k += 1
                        zr = wk.tile([P, 8], F32, tag="zr")
                        nc.vector.tensor_scalar_add(zr[:], agg[:, 256:264], 1e-16)
                        nc.vector.reciprocal(zr[:], zr[:])
                        if layer == 1:
                            zrb = zr[:].unsqueeze(1).to_broadcast([P, 32, H])
                            hn = wk.tile([P, 256], F32, tag="hn")
                            nc.vector.tensor_tensor(
                                out=hn[:].rearrange("p (f h) -> p f h", h=H),
                                in0=agg[:, 0:256].rearrange("p (f h) -> p f h", h=H),
                                in1=zrb, op=ALU.mult)
                            t2t = wk.tile([P, 64], F32, tag="t2t")
                            nc.vector.memset(t2t[:], 0.0)
                            nc.vector.tensor_reduce(
                                out=t2t[:, 0:32],
                                in_=hn[:].rearrange("p (f h) -> p f h", h=H),
                                axis=AX.X, op=ALU.add)
                            nc.scalar.mul(t2t[:, 0:32], t2t[:, 0:32], 1.0 / H)
                            nc.vector.tensor_tensor(out=t2t[:, 0:32], in0=t2t[:, 0:32],
                                                    in1=bias_b[:], op=ALU.add)
                            nc.vector.tensor_scalar_max(t2t[:, 0:32], t2t[:, 0:32], 0.0)
                            hT = pso.tile([32, P], F32, tag="hT")
                            nc.tensor.transpose(hT[:], t2t[:, 0:32], ident[:])
                            hTs = wk.tile([32, P], F32, tag="hTs")
                            nc.vector.tensor_copy(hTs[:], hT[:])
                            asd = pso.tile([P, 16], F32, tag="asd")
                            nc.tensor.matmul(out=asd[:], lhsT=hTs[:], rhs=wt2[:],
                                             start=True, stop=True)
                            nc.vector.tensor_copy(t2t[:, 32:40], asd[:, 0:8])
                            nc.sync.dma_start(T2o[w * P:w * P + r, :], t2t[:r, :])
                            ad2t = wk.tile([P, 64], F32, tag="ad2t")
                            nc.vector.memset(ad2t[:], 0.0)
                            nc.vector.tensor_copy(ad2t[:, 0:8], asd[:, 8:16])
                            nc.sync.dma_start(AD2o[w * P:w * P + r, :], ad2t[:r, :])
                        else:
                            zrb = zr[:].unsqueeze(2).to_broadcast([P, H, 32])
                            anb = wk.tile([P, 256], BF16, tag="anb")
                            nc.vector.tensor_tensor(
                                out=anb[:].rearrange("p (h f) -> p h f", h=H),
                                in0=agg[:, 0:256].rearrange("p (h f) -> p h f", h=H),
                                in1=zrb, op=ALU.mult)
                            o2 = pso.tile([P, 16], F32, tag="o2")
                            for half in range(2):
                                tps = pst.tile([P, P], BF16, tag="tps")
                                nc.tensor.transpose(
                                    tps[:], anb[:, half * 128:(half + 1) * 128],
                                    ident_bf[:])
                                tsb = wk.tile([P, P], BF16, tag="tsb")
                                nc.vector.tensor_copy(tsb[:], tps[:])
                                nc.tensor.matmul(out=o2[:], lhsT=tsb[:],
                                                 rhs=(w2r0 if half == 0 else w2r1)[:],
                                                 start=(half == 0), stop=(half == 1))
                            ot = wk.tile([P, 16], F32, tag="ot")
                            nc.scalar.mul(ot[:], o2[:], 1.0 / H)
                            nc.vector.tensor_tensor(out=ot[:], in0=ot[:], in1=bias_b[:],
                                                    op=ALU.add)
                            nc.sync.dma_start(OUT[w * P:w * P + r, :], ot[:r, :])
    nc.compile()
    return nc, names


# ---------------- driver ----------------


def _perm_fh():
    """column permutation: (h*32+f) -> (f*8+h)"""
    perm = np.empty(256, np.int64)
    for f in range(32):
        for h in range(H):
            perm[f * H + h] = h * 32 + f
    return perm


def _run_pipeline(inputs, dims, trace=False):
    x = np.ascontiguousarray(np.asarray(inputs['x'], np.float32))
    ei = np.asarray(inputs['edge_index'])
    W1 = np.ascontiguousarray(np.asarray(inputs['W1'], np.float32))
    as1 = np.asarray(inputs['att_src1'], np.float32)
    ad1 = np.asarray(inputs['att_dst1'], np.float32)
    b1 = np.asarray(inputs['b1'], np.float32)
    W2 = np.ascontiguousarray(np.asarray(inputs['W2'], np.float32))
    as2 = np.asarray(inputs['att_src2'], np.float32)
    ad2 = np.asarray(inputs['att_dst2'], np.float32)
    b2 = np.asarray(inputs['b2'], np.float32)
    NC, NPC = dims.NCORES, dims.NPC

    plan, streams = build_plan(ei, dims)
    times = {}

    perm = _perm_fh()
    W1p = np.ascontiguousarray(W1[:, perm])
    as1p = as1.reshape(-1)[perm]
    ad1p = ad1.reshape(-1)[perm]

    nc1, n1 = build_dense1(dims)
    att1 = np.ascontiguousarray(
        np.concatenate([as1p, ad1p]).reshape(1, -1).astype(np.float32))
    xTb = np.ascontiguousarray(x.T.astype(ml_dtypes.bfloat16))
    ins1 = [{n1['xT']: np.ascontiguousarray(xTb[:, c * NPC:(c + 1) * NPC]),
             n1['W1']: W1p, n1['att1']: att1} for c in range(NC)]
    r1 = bass_utils.run_bass_kernel_spmd(nc1, ins1, core_ids=list(range(NC)),
                                         trace=trace)
    times['dense1'] = r1.exec_time_ns
    T1full = np.concatenate([r1.results[c][n1['T1']] for c in range(NC)])
    ad1_shards = [r1.results[c][n1['AD1']] for c in range(NC)]

    nc2, n2 = build_edge(1, plan, dims)
    att2 = np.ascontiguousarray(np.concatenate(
        [as2.reshape(-1), ad2.reshape(-1)]).reshape(1, -1).astype(np.float32))
    ins2 = [{n2['G']: T1full, n2['AD']: ad1_shards[c], n2['W2']: W2,
             n2['att2']: att2,
             n2['bias']: np.ascontiguousarray(b1.reshape(1, -1)),
             n2['gidx']: streams[c]['gidx'], n2['dcidx']: streams[c]['dcidx'],
             n2['dstloc']: streams[c]['dstloc']} for c in range(NC)]
    r2 = bass_utils.run_bass_kernel_spmd(nc2, ins2, core_ids=list(range(NC)),
                                         trace=trace)
    times['edge1'] = r2.exec_time_ns
    T2full = np.concatenate([r2.results[c][n2['T2']] for c in range(NC)])
    ad2_shards = []
    for c in range(NC):
        a = r2.results[c][n2['AD2']].copy()
        a[NPC, :] = POISON
        ad2_shards.append(a)

    nc3, n3 = build_edge(2, plan, dims)
    ins3 = [{n3['G']: T2full, n3['AD']: ad2_shards[c], n3['W2']: W2,
             n3['bias']: np.ascontiguousarray(b2.reshape(1, -1)),
             n3['gidx']: streams[c]['gidx'], n3['dcidx']: streams[c]['dcidx'],
             n3['dstloc']: streams[c]['dstloc']} for c in range(NC)]
    r3 = bass_utils.run_bass_kernel_spmd(nc3, ins3, core_ids=list(range(NC)),
                                         trace=trace)
    times['edge2'] = r3.exec_time_ns
    out = np.concatenate([r3.results[c][n3['OUT']] for c in range(NC)])
    return np.ascontiguousarray(out.astype(np.float32)), times


def kernel(**inputs):
    out, _ = _run_pipeline(inputs, Dims(), trace=False)
    return out


# revision 33
# speedup vs baseline: 1.1885x; 1.1885x over previous
"""Self-contained Trainium2 Bass kernel for the 2-layer GAT
(nn_GAT_18915035971953): 100000 nodes, 1.6M edges, 8 NeuronCores.

Strategy: edges sorted by destination and dst-sharded across 8 cores
(12500 dst nodes each); per 128-dst window, per-edge source rows are
fetched with dma_gather (4 SWDGE queues), edge softmax weights are
computed on-chip, and the segment sum is a one-hot matmul into PSUM.
Three SPMD launches: dense layer-1 tables -> layer-1 edge phase ->
layer-2 edge phase; the host only reorders indices and concatenates
shard outputs between launches.
"""
import sys, types
from dataclasses import dataclass
import numpy as np
import ml_dtypes

if "/opt/trn_rl_repo" not in sys.path:
    sys.path.insert(0, "/opt/trn_rl_repo")

import concourse.bacc as bacc
import concourse.mybir as mybir
import concourse.tile as tile
from concourse.masks import make_identity
from concourse import bass_utils

# ---------------- host-side index preprocessing ----------------


P = 128


@dataclass
class Dims:
    N: int = 100000          # total nodes
    NCORES: int = 8
    NBUCK: int = 4           # src buckets (int16 gather indices < 32768)
    MAXNIDX: int = 1024      # max idxs per dma_gather call

    @property
    def NPC(self):
        return self.N // self.NCORES

    @property
    def BUCK(self):
        return self.N // self.NBUCK


def _wrap16(idx):
    n = idx.shape[0]
    assert n % 16 == 0
    w = idx.reshape(n // 16, 16).T.astype(np.int16)
    return np.tile(w, (8, 1))


def build_uniform_plans(edge_index, dims: Dims):
    """Returns (plan, streams).
    plan: dict(nwin, windows=[{S, slot0, calls=[(bucket, n, gcol0)]}], slots, gcols)
    streams (per core): gidx int16 [128, gcols], dcidx int16 [128, slots*8],
    dstloc bf16 [128, slots].
    Pad slots: gather row 0 of the bucket (finite data), dc idx -> poison row NPC
    (a_d = -1e30 -> w = 0), dstloc = 0.
    """
    N, NC, NB, BUCK = dims.N, dims.NCORES, dims.NBUCK, dims.BUCK
    NPC = dims.NPC
    src = np.asarray(edge_index[0], np.int64)
    dst = np.asarray(edge_index[1], np.int64)
    order = np.argsort(dst, kind="stable")
    s_src, s_dst = src[order], dst[order]
    counts = np.bincount(s_dst, minlength=N)
    node_start = np.concatenate([[0], np.cumsum(counts)])

    nwin = (NPC + P - 1) // P
    seg = [[[None] * NB for _ in range(nwin)] for _ in range(NC)]
    for c in range(NC):
        d0 = c * NPC
        for w in range(nwin):
            lo = node_start[d0 + w * P]
            hi = node_start[min(d0 + (w + 1) * P, d0 + NPC)]
            esrc = s_src[lo:hi]
            edst = s_dst[lo:hi]
            for b in range(NB):
                m = (esrc // BUCK) == b
                seg[c][w][b] = (esrc[m], edst[m])

    windows = []
    gcol0 = 0
    slot0 = 0
    core_g = [[] for _ in range(NC)]
    core_dc = [[] for _ in range(NC)]
    core_dl = [[] for _ in range(NC)]
    for w in range(nwin):
        calls = []
        for b in range(NB):
            nmax = max(seg[c][w][b][0].shape[0] for c in range(NC))
            if nmax == 0:
                continue
            nn = ((nmax + P - 1) // P) * P
            calls.append((b, nn, gcol0))
            gcol0 += nn // 16
            for c in range(NC):
                es, ed = seg[c][w][b]
                k = es.shape[0]
                d0 = c * NPC
                gi = np.concatenate([es - b * BUCK, np.zeros(nn - k, np.int64)])
                dc = np.concatenate([ed - d0, np.full(nn - k, NPC, np.int64)])
                dl = np.concatenate([ed - (d0 + w * P), np.full(nn - k, -1, np.int64)])
                core_g[c].append(_wrap16(gi))
                core_dc[c].append(_wrap16(dc))
                core_dl[c].append(dl.reshape(nn // P, P).T)
        if not calls:
            calls.append((0, P, gcol0))
            gcol0 += P // 16
            for c in range(NC):
                core_g[c].append(_wrap16(np.zeros(P, np.int64)))
                core_dc[c].append(_wrap16(np.full(P, NPC, np.int64)))
                core_dl[c].append(np.full((P, 1), -1, np.int64))
        S = sum(nn // P for (_, nn, _) in calls)
        windows.append(dict(S=S, slot0=slot0, calls=calls))
        slot0 += S

    plan = dict(nwin=nwin, windows=windows, slots=slot0, gcols=gcol0)
    streams = []
    for c in range(NC):
        streams.append(dict(
            gidx=np.ascontiguousarray(np.concatenate(core_g[c], axis=1)),
            dcidx=np.ascontiguousarray(np.concatenate(core_dc[c], axis=1)),
            dstloc=np.ascontiguousarray(
                np.concatenate(core_dl[c], axis=1).astype(ml_dtypes.bfloat16)),
        ))
    return plan, streams


# ---------------- kernel builders ----------------


P = 128
H = 8
POISON = -1.0e30
MAXNIDX = 1024
NQ = 4
USE_PT_LAYERS = (1,)     # a_d via PT-transpose matmul instead of dc-gather
F32 = mybir.dt.float32
BF16 = mybir.dt.bfloat16
I16 = mybir.dt.int16
AF = mybir.ActivationFunctionType
ALU = mybir.AluOpType
AX = mybir.AxisListType


def build_dense1(dims):
    NPC = dims.NPC
    nc = bacc.Bacc(None, target_bir_lowering=False, num_swdge_queues=NQ)
    with tile.TileContext(nc) as tc:
        with tc.tile_pool(name="dram", bufs=1, space="DRAM") as dram:
            xT = dram.tile([P, NPC], BF16, kind="ExternalInput")
            W1 = dram.tile([P, 256], F32, kind="ExternalInput")
            att1 = dram.tile([1, 512], F32, kind="ExternalInput")
            T1 = dram.tile([NPC, 384], BF16, kind="ExternalOutput")
            AD1 = dram.tile([NPC + 1, 64], F32, kind="ExternalOutput")
            names = dict(xT=xT.name, W1=W1.name, att1=att1.name,
                         T1=T1.name, AD1=AD1.name)
            with tc.tile_pool(name="cst", bufs=1) as cst, \
                 tc.tile_pool(name="wk", bufs=3) as wk, \
                 tc.tile_pool(name="ps", bufs=4, space="PSUM") as ps:
                xTs = cst.tile([P, NPC], BF16)
                nc.sync.dma_start(xTs[:], xT[:])
                rhs = cst.tile([P, 272], F32)
                nc.sync.dma_start(rhs[:, 0:256], W1[:])
                att_s = cst.tile([1, 512], F32)
                nc.sync.dma_start(att_s[:], att1[:])
                attb = cst.tile([P, 512], F32)
                nc.gpsimd.partition_broadcast(attb[:, 0:256], att_s[0:1, 0:256])
                nc.gpsimd.partition_broadcast(attb[:, 256:512], att_s[0:1, 256:512])
                tmp = cst.tile([P, 512], F32)
                nc.vector.tensor_tensor(out=tmp[:, 0:256], in0=rhs[:, 0:256],
                                        in1=attb[:, 0:256], op=ALU.mult)
                nc.vector.tensor_tensor(out=tmp[:, 256:512], in0=rhs[:, 0:256],
                                        in1=attb[:, 256:512], op=ALU.mult)
                tv = tmp[:].rearrange("p (v h f) -> p v h f", v=2, h=H)
                nc.vector.tensor_reduce(out=rhs[:, 256:264], in_=tv[:, 0],
                                        axis=AX.X, op=ALU.add)
                nc.vector.tensor_reduce(out=rhs[:, 264:272], in_=tv[:, 1],
                                        axis=AX.X, op=ALU.add)
                rhs_bf = cst.tile([P, 272], BF16)
                nc.vector.tensor_copy(rhs_bf[:], rhs[:])
                ntile = (NPC + P - 1) // P
                for i in range(ntile):
                    r = min(P, NPC - i * P)
                    po = ps.tile([P, 272], F32, tag="po")
                    nc.tensor.matmul(out=po[:r, :], lhsT=xTs[:, i * P:i * P + r],
                                     rhs=rhs_bf[:], start=True, stop=True)
                    t1t = wk.tile([P, 384], BF16, tag="t1t")
                    nc.vector.memset(t1t[:], 0.0)
                    nc.scalar.copy(t1t[:r, 0:256], po[:r, 0:256])
                    nc.vector.tensor_copy(t1t[:, 256:384].bitcast(F32)[:r, 0:8],
                                          po[:r, 256:264])
                    nc.sync.dma_start(T1[i * P:i * P + r, :], t1t[:r, :])
                    adt = wk.tile([P, 64], F32, tag="adt")
                    nc.vector.memset(adt[:], 0.0)
                    nc.vector.tensor_copy(adt[:r, 0:8], po[:r, 264:272])
                    nc.sync.dma_start(AD1[i * P:i * P + r, :], adt[:r, :])
                pois = wk.tile([1, 64], F32, tag="pois")
                nc.vector.memset(pois[:], POISON)
                nc.sync.dma_start(AD1[NPC:NPC + 1, :], pois[:])
    nc.compile()
    return nc, names


def _split(calls, maxn):
    out = []
    scol = 0
    for (b, n, col0) in calls:
        off = 0
        while off < n:
            take = min(maxn, n - off)
            out.append((b, take, col0 + off // 16, scol))
            scol += take // P
            off += take
    return out


def build_edge(layer, plan, dims):
    N, NPC, BUCK, NB = dims.N, dims.NPC, dims.BUCK, dims.NBUCK
    windows = plan['windows']
    slots = plan['slots']
    gcols = plan['gcols']
    GW = 384 if layer == 1 else 64          # gather row width (elements)
    GDT = BF16 if layer == 1 else F32
    nc = bacc.Bacc(None, target_bir_lowering=False, num_swdge_queues=NQ)
    USE_PT = layer in USE_PT_LAYERS
    qctr = [0]

    def nextq():
        q = qctr[0] % NQ
        qctr[0] += 1
        return q

    with tile.TileContext(nc) as tc:
        with tc.tile_pool(name="dram", bufs=1, space="DRAM") as dram:
            names = {}
            Gt = dram.tile([N, GW], GDT, kind="ExternalInput")
            ADt = dram.tile([NPC + 1, 64], F32, kind="ExternalInput")
            W2 = dram.tile([32, 128], F32, kind="ExternalInput")
            nb = 32 if layer == 1 else 16
            bias = dram.tile([1, nb], F32, kind="ExternalInput")
            names.update(G=Gt.name, AD=ADt.name, W2=W2.name, bias=bias.name)
            if layer == 1:
                att2 = dram.tile([1, 256], F32, kind="ExternalInput")
                T2o = dram.tile([NPC, 64], F32, kind="ExternalOutput")
                AD2o = dram.tile([NPC + 1, 64], F32, kind="ExternalOutput")
                names.update(att2=att2.name, T2=T2o.name, AD2=AD2o.name)
            else:
                OUT = dram.tile([NPC, 16], F32, kind="ExternalOutput")
                names.update(OUT=OUT.name)
            gidx = dram.tile([P, gcols], I16, kind="ExternalInput")
            dcidx = dram.tile([P, slots * 8], I16, kind="ExternalInput")
            dstloc = dram.tile([P, slots], BF16, kind="ExternalInput")
            names.update(gidx=gidx.name, dcidx=dcidx.name, dstloc=dstloc.name)

            with tc.tile_pool(name="cst", bufs=1) as cst, \
                 tc.tile_pool(name="gp", bufs=2) as gp, \
                 tc.tile_pool(name="wk", bufs=2) as wk, \
                 tc.tile_pool(name="psa", bufs=2, space="PSUM") as psa, \
                 tc.tile_pool(name="pst", bufs=2, space="PSUM") as pst, \
                 tc.tile_pool(name="pso", bufs=2, space="PSUM") as pso, \
                 tc.tile_pool(name="psd", bufs=2, space="PSUM") as psd:
                gidx_s = cst.tile([P, gcols], I16)
                nc.sync.dma_start(gidx_s[:], gidx[:])
                dcidx_s = cst.tile([P, slots * 8], I16)
                nc.sync.dma_start(dcidx_s[:], dcidx[:])
                dstloc_s = cst.tile([P, slots], BF16)
                nc.sync.dma_start(dstloc_s[:], dstloc[:])
                iota_i = cst.tile([P, P], mybir.dt.int32)
                nc.gpsimd.iota(iota_i[:], pattern=[[1, P]], base=0,
                               channel_multiplier=0)
                iota_bf = cst.tile([P, P], BF16)
                nc.vector.tensor_copy(iota_bf[:], iota_i[:])
                bias_s = cst.tile([1, nb], F32)
                nc.sync.dma_start(bias_s[:], bias[:])
                bias_b = cst.tile([P, nb], F32)
                nc.gpsimd.partition_broadcast(bias_b[:], bias_s[0:1, :])
                W2s = cst.tile([32, 128], F32)
                nc.sync.dma_start(W2s[:], W2[:])
                ident_bf = cst.tile([P, P], BF16)
                make_identity(nc, ident_bf[:])
                if layer == 1:
                    att2_s = cst.tile([1, 256], F32)
                    nc.sync.dma_start(att2_s[:], att2[:])
                    att2b = cst.tile([32, 256], F32)
                    nc.gpsimd.partition_broadcast(att2b[:, 0:128], att2_s[0:1, 0:128])
                    nc.gpsimd.partition_broadcast(att2b[:, 128:256], att2_s[0:1, 128:256])
                    tmp2 = cst.tile([32, 256], F32)
                    nc.vector.tensor_tensor(out=tmp2[:, 0:128], in0=W2s[:],
                                            in1=att2b[:, 0:128], op=ALU.mult)
                    nc.vector.tensor_tensor(out=tmp2[:, 128:256], in0=W2s[:],
                                            in1=att2b[:, 128:256], op=ALU.mult)
                    t2v = tmp2[:].rearrange("p (v h f) -> p v h f", v=2, h=H)
                    wt2 = cst.tile([32, 16], F32)
                    nc.vector.tensor_reduce(out=wt2[:, 0:8], in_=t2v[:, 0],
                                            axis=AX.X, op=ALU.add)
                    nc.vector.tensor_reduce(out=wt2[:, 8:16], in_=t2v[:, 1],
                                            axis=AX.X, op=ALU.add)
                    ident = cst.tile([P, P], F32)
                    make_identity(nc, ident[:])
                else:
                    w2r0 = cst.tile([P, 16], BF16)
                    w2r1 = cst.tile([P, 16], BF16)
                    for h in range(H):
                        dst = w2r0 if h < 4 else w2r1
                        nc.vector.tensor_copy(
                            dst[(h % 4) * 32:(h % 4 + 1) * 32, :],
                            W2s[:, h * 16:(h + 1) * 16])

                for w, win in enumerate(windows):
                    S = win['S']
                    r = min(P, NPC - w * P)
                    g_t = gp.tile([P, S, GW], GDT, tag="g")
                    for (b, n, gc0, scol) in _split(win['calls'], MAXNIDX):
                        nc.gpsimd.dma_gather(
                            g_t[:, scol:scol + n // P, :],
                            Gt[b * BUCK:(b + 1) * BUCK, :],
                            gidx_s[:, gc0:gc0 + n // 16], n, n, GW,
                            queue_num=nextq())
                    if not USE_PT:
                        dc_t = gp.tile([P, S, 64], F32, tag="dc")
                        for (_, n, dc0, scol) in _split(
                                [(0, S * P, win['slot0'] * 8)], MAXNIDX):
                            nc.gpsimd.dma_gather(
                                dc_t[:, scol:scol + n // P, :], ADt[:],
                                dcidx_s[:, dc0:dc0 + n // 16], n, n, 64,
                                queue_num=nextq())
                    if layer == 1:
                        a_s_ap = g_t[:].rearrange("p s e -> p (s e)") \
                            .bitcast(F32).rearrange("p (s e) -> p s e", e=192)[:, :, 128:136]
                        feats = g_t[:, :, 0:256]
                    else:
                        a_s_ap = g_t[:, :, 32:40]
                        gb_t = wk.tile([P, S, 32], BF16, tag="gb")
                        nc.vector.tensor_copy(gb_t[:], g_t[:, :, 0:32])
                        feats = gb_t[:]
                    p_t = wk.tile([P, S, P], BF16, tag="pt")
                    dl_b = dstloc_s[:, win['slot0']:win['slot0'] + S] \
                        .unsqueeze(2).to_broadcast([P, S, P])
                    io_b = iota_bf[:].unsqueeze(1).to_broadcast([P, S, P])
                    nc.vector.tensor_tensor(out=p_t[:], in0=dl_b, in1=io_b,
                                            op=ALU.is_equal)
                    if USE_PT:
                        adw = wk.tile([P, 8], F32, tag="adw")
                        nc.vector.memset(adw[:], 0.0)
                        nc.sync.dma_start(adw[:r, :], ADt[w * P:w * P + r, 0:8])
                        adw_b = wk.tile([P, 8], BF16, tag="adwb")
                        nc.vector.tensor_copy(adw_b[:], adw[:])
                        adE = psd.tile([P, S * 8], F32, tag="adE")
                        for s in range(S):
                            ptp = pst.tile([P, P], BF16, tag="tps")
                            nc.tensor.transpose(ptp[:], p_t[:, s, :], ident_bf[:])
                            pts = wk.tile([P, P], BF16, tag="pts")
                            nc.scalar.copy(pts[:], ptp[:])
                            nc.tensor.matmul(out=adE[:, s * 8:(s + 1) * 8],
                                             lhsT=pts[:], rhs=adw_b[:],
                                             start=True, stop=True)
                        ad_ap = adE[:].rearrange("p (s e) -> p s e", e=8)
                    else:
                        ad_ap = dc_t[:, :, 0:8]
                    et = wk.tile([P, S, 8], F32, tag="et")
                    nc.vector.tensor_tensor(out=et[:], in0=a_s_ap,
                                            in1=ad_ap, op=ALU.add)
                    nc.vector.scalar_tensor_tensor(
                        out=et[:], in0=et[:], scalar=0.2, in1=et[:],
                        op0=ALU.mult, op1=ALU.max)
                    rhs_t = wk.tile([P, S, 264], BF16, tag="rhs")
                    nc.scalar.activation(rhs_t[:, :, 256:264], et[:], AF.Exp)
                    wexp_b = rhs_t[:, :, 256:264].unsqueeze(3) \
                        .to_broadcast([P, S, 8, 32])
                    if layer == 1:
                        g_v = feats.rearrange("p s (h f) -> p s h f", h=H)
                    else:
                        g_v = feats.unsqueeze(2).to_broadcast([P, S, 8, 32])
                    nc.vector.tensor_tensor(
                        out=rhs_t[:, :, 0:256].rearrange("p s (h f) -> p s h f", h=H),
                        in0=g_v, in1=wexp_b, op=ALU.mult)
                    agg = psa.tile([P, 264], F32, tag="agg")
                    for s in range(S):
                        nc.tensor.matmul(out=agg[:], lhsT=p_t[:, s, :],
                                         rhs=rhs_t[:, s, :],
                                         start=(s == 0), stop=(s == S - 1))
                    zr = wk.tile([P, 8], F32, tag="zr")
                    nc.vector.tensor_scalar_add(zr[:], agg[:, 256:264], 1e-16)
                    nc.vector.reciprocal(zr[:], zr[:])
                    zrb = zr[:].unsqueeze(2).to_broadcast([P, H, 32])
                    if layer == 1:
                        hn = wk.tile([P, 256], F32, tag="hn")
                        nc.vector.tensor_tensor(
                            out=hn[:].rearrange("p (h f) -> p h f", h=H),
                            in0=agg[:, 0:256].rearrange("p (h f) -> p h f", h=H),
                            in1=zrb, op=ALU.mult)
                        t2t = wk.tile([P, 64], F32, tag="t2t")
                        nc.vector.memset(t2t[:], 0.0)
                        nc.vector.tensor_reduce(
                            out=t2t[:, 0:32],
                            in_=hn[:].rearrange("p (h f) -> p f h", h=H),
                            axis=AX.X, op=ALU.add)
                        nc.scalar.mul(t2t[:, 0:32], t2t[:, 0:32], 1.0 / H)
                        nc.vector.tensor_tensor(out=t2t[:, 0:32], in0=t2t[:, 0:32],
                                                in1=bias_b[:], op=ALU.add)
                        nc.vector.tensor_scalar_max(t2t[:, 0:32], t2t[:, 0:32], 0.0)
                        hT = pso.tile([32, P], F32, tag="asd")
                        nc.tensor.transpose(hT[:], t2t[:, 0:32], ident[:])
                        hTs = wk.tile([32, P], F32, tag="hTs")
                        nc.vector.tensor_copy(hTs[:], hT[:])
                        asd = pso.tile([P, 16], F32, tag="asd")
                        nc.tensor.matmul(out=asd[:], lhsT=hTs[:], rhs=wt2[:],
                                         start=True, stop=True)
                        nc.vector.tensor_copy(t2t[:, 32:40], asd[:, 0:8])
                        nc.sync.dma_start(T2o[w * P:w * P + r, :], t2t[:r, :])
                        ad2t = wk.tile([P, 64], F32, tag="ad2t")
                        nc.vector.memset(ad2t[:], 0.0)
                        nc.vector.tensor_copy(ad2t[:, 0:8], asd[:, 8:16])
                        nc.sync.dma_start(AD2o[w * P:w * P + r, :], ad2t[:r, :])
                    else:
                        anb = wk.tile([P, 256], BF16, tag="anb")
                        nc.vector.tensor_tensor(
                            out=anb[:].rearrange("p (h f) -> p h f", h=H),
                            in0=agg[:, 0:256].rearrange("p (h f) -> p h f", h=H),
                            in1=zrb, op=ALU.mult)
                        o2 = pso.tile([P, 16], F32, tag="o2")
                        for half in range(2):
                            tps = pst.tile([P, P], BF16, tag="tps")
                            nc.tensor.transpose(
                                tps[:], anb[:, half * 128:(half + 1) * 128],
                                ident_bf[:])
                            tsb = wk.tile([P, P], BF16, tag="tsb")
                            nc.vector.tensor_copy(tsb[:], tps[:])
                            nc.tensor.matmul(out=o2[:], lhsT=tsb[:],
                                             rhs=(w2r0 if half == 0 else w2r1)[:],
                                             start=(half == 0), stop=(half == 1))
                        ot = wk.tile([P, 16], F32, tag="ot")
                        nc.scalar.mul(ot[:], o2[:], 1.0 / H)
                        nc.vector.tensor_tensor(out=ot[:], in0=ot[:], in1=bias_b[:],
                                                op=ALU.add)
                        nc.sync.dma_start(OUT[w * P:w * P + r, :], ot[:r, :])
    nc.compile()
    return nc, names


# ---------------- driver ----------------


def _run_pipeline(inputs, dims, trace=False):
    x = np.ascontiguousarray(np.asarray(inputs['x'], np.float32))
    ei = np.asarray(inputs['edge_index'])
    W1 = np.ascontiguousarray(np.asarray(inputs['W1'], np.float32))
    as1 = np.asarray(inputs['att_src1'], np.float32)
    ad1 = np.asarray(inputs['att_dst1'], np.float32)
    b1 = np.asarray(inputs['b1'], np.float32)
    W2 = np.ascontiguousarray(np.asarray(inputs['W2'], np.float32))
    as2 = np.asarray(inputs['att_src2'], np.float32)
    ad2 = np.asarray(inputs['att_dst2'], np.float32)
    b2 = np.asarray(inputs['b2'], np.float32)
    NC, NPC = dims.NCORES, dims.NPC

    plan, streams = build_uniform_plans(ei, dims)
    times = {}

    nc1, n1 = build_dense1(dims)
    att1 = np.ascontiguousarray(np.concatenate(
        [as1.reshape(-1), ad1.reshape(-1)]).reshape(1, -1).astype(np.float32))
    xTb = np.ascontiguousarray(x.T.astype(ml_dtypes.bfloat16))
    ins1 = [{n1['xT']: np.ascontiguousarray(xTb[:, c * NPC:(c + 1) * NPC]),
             n1['W1']: W1, n1['att1']: att1} for c in range(NC)]
    r1 = bass_utils.run_bass_kernel_spmd(nc1, ins1, core_ids=list(range(NC)),
                                         trace=trace)
    times['dense1'] = r1.exec_time_ns
    T1full = np.concatenate([r1.results[c][n1['T1']] for c in range(NC)])
    ad1_shards = [r1.results[c][n1['AD1']] for c in range(NC)]

    nc2, n2 = build_edge(1, plan, dims)
    att2 = np.ascontiguousarray(np.concatenate(
        [as2.reshape(-1), ad2.reshape(-1)]).reshape(1, -1).astype(np.float32))
    ins2 = [{n2['G']: T1full, n2['AD']: ad1_shards[c], n2['W2']: W2,
             n2['att2']: att2,
             n2['bias']: np.ascontiguousarray(b1.reshape(1, -1)),
             n2['gidx']: streams[c]['gidx'], n2['dcidx']: streams[c]['dcidx'],
             n2['dstloc']: streams[c]['dstloc']} for c in range(NC)]
    r2 = bass_utils.run_bass_kernel_spmd(nc2, ins2, core_ids=list(range(NC)),
                                         trace=trace)
    times['edge1'] = r2.exec_time_ns
    T2full = np.concatenate([r2.results[c][n2['T2']] for c in range(NC)])
    ad2_shards = []
    for c in range(NC):
        a = r2.results[c][n2['AD2']].copy()
        a[NPC, :] = POISON
        ad2_shards.append(a)

    nc3, n3 = build_edge(2, plan, dims)
    ins3 = [{n3['G']: T2full, n3['AD']: ad2_shards[c], n3['W2']: W2,
             n3['bias']: np.ascontiguousarray(b2.reshape(1, -1)),
             n3['gidx']: streams[c]['gidx'], n3['dcidx']: streams[c]['dcidx'],
             n3['dstloc']: streams[c]['dstloc']} for c in range(NC)]
    r3 = bass_utils.run_bass_kernel_spmd(nc3, ins3, core_ids=list(range(NC)),
                                         trace=trace)
    times['edge2'] = r3.exec_time_ns
    out = np.concatenate([r3.results[c][n3['OUT']] for c in range(NC)])
    return np.ascontiguousarray(out.astype(np.float32)), times


def kernel(**inputs):
    out, _ = _run_pipeline(inputs, Dims(), trace=False)
    return out


# revision 42
# speedup vs baseline: 1.3818x; 1.1627x over previous
"""Self-contained Trainium2 Bass kernel for the 2-layer GAT
(nn_GAT_18915035971953): 100000 nodes, 1.6M edges, 8 NeuronCores.

Strategy: edges sorted by destination and dst-sharded across 8 cores
(12500 dst nodes each); per 128-dst window, per-edge source rows are
fetched with dma_gather (4 SWDGE queues), edge softmax weights are
computed on-chip, and the segment sum is a one-hot matmul into PSUM.
Three SPMD launches: dense layer-1 tables -> layer-1 edge phase ->
layer-2 edge phase; the host only reorders indices and concatenates
shard outputs between launches.
"""
import sys, types
from dataclasses import dataclass
import numpy as np
import ml_dtypes

if "/opt/trn_rl_repo" not in sys.path:
    sys.path.insert(0, "/opt/trn_rl_repo")

import concourse.bacc as bacc
import concourse.mybir as mybir
import concourse.tile as tile
from concourse.masks import make_identity
from concourse import bass_utils

# ---------------- host-side index preprocessing ----------------


P = 128


@dataclass
class Dims:
    N: int = 100000          # total nodes
    NCORES: int = 8
    NBUCK: int = 4           # src buckets (int16 gather indices < 32768)
    MAXNIDX: int = 1024      # max idxs per dma_gather call

    @property
    def NPC(self):
        return self.N // self.NCORES

    @property
    def BUCK(self):
        return self.N // self.NBUCK


def _wrap16(idx):
    n = idx.shape[0]
    assert n % 16 == 0
    w = idx.reshape(n // 16, 16).T.astype(np.int16)
    return np.tile(w, (8, 1))


def build_uniform_plans(edge_index, dims: Dims):
    """Returns (plan, streams).
    plan: dict(nwin, windows=[{S, slot0, calls=[(bucket, n, gcol0)]}], slots, gcols)
    streams (per core): gidx int16 [128, gcols], dcidx int16 [128, slots*8],
    dstloc bf16 [128, slots].
    Pad slots: gather row 0 of the bucket (finite data), dc idx -> poison row NPC
    (a_d = -1e30 -> w = 0), dstloc = 0.
    """
    N, NC, NB, BUCK = dims.N, dims.NCORES, dims.NBUCK, dims.BUCK
    NPC = dims.NPC
    src = np.asarray(edge_index[0], np.int64)
    dst = np.asarray(edge_index[1], np.int64)
    order = np.argsort(dst, kind="stable")
    s_src, s_dst = src[order], dst[order]
    counts = np.bincount(s_dst, minlength=N)
    node_start = np.concatenate([[0], np.cumsum(counts)])

    nwin = (NPC + P - 1) // P
    seg = [[[None] * NB for _ in range(nwin)] for _ in range(NC)]
    for c in range(NC):
        d0 = c * NPC
        for w in range(nwin):
            lo = node_start[d0 + w * P]
            hi = node_start[min(d0 + (w + 1) * P, d0 + NPC)]
            esrc = s_src[lo:hi]
            edst = s_dst[lo:hi]
            for b in range(NB):
                m = (esrc // BUCK) == b
                seg[c][w][b] = (esrc[m], edst[m])

    windows = []
    gcol0 = 0
    slot0 = 0
    core_g = [[] for _ in range(NC)]
    core_dc = [[] for _ in range(NC)]
    core_dl = [[] for _ in range(NC)]
    for w in range(nwin):
        calls = []
        for b in range(NB):
            nmax = max(seg[c][w][b][0].shape[0] for c in range(NC))
            if nmax == 0:
                continue
            nn = ((nmax + P - 1) // P) * P
            calls.append((b, nn, gcol0))
            gcol0 += nn // 16
            for c in range(NC):
                es, ed = seg[c][w][b]
                k = es.shape[0]
                d0 = c * NPC
                gi = np.concatenate([es - b * BUCK, np.zeros(nn - k, np.int64)])
                dc = np.concatenate([ed - d0, np.full(nn - k, NPC, np.int64)])
                dl = np.concatenate([ed - (d0 + w * P), np.full(nn - k, -1, np.int64)])
                core_g[c].append(_wrap16(gi))
                core_dc[c].append(_wrap16(dc))
                core_dl[c].append(dl.reshape(nn // P, P).T)
        if not calls:
            calls.append((0, P, gcol0))
            gcol0 += P // 16
            for c in range(NC):
                core_g[c].append(_wrap16(np.zeros(P, np.int64)))
                core_dc[c].append(_wrap16(np.full(P, NPC, np.int64)))
                core_dl[c].append(np.full((P, 1), -1, np.int64))
        S = sum(nn // P for (_, nn, _) in calls)
        windows.append(dict(S=S, slot0=slot0, calls=calls))
        slot0 += S

    plan = dict(nwin=nwin, windows=windows, slots=slot0, gcols=gcol0)
    streams = []
    for c in range(NC):
        streams.append(dict(
            gidx=np.ascontiguousarray(np.concatenate(core_g[c], axis=1)),
            dcidx=np.ascontiguousarray(np.concatenate(core_dc[c], axis=1)),
            dstloc=np.ascontiguousarray(
                np.concatenate(core_dl[c], axis=1).astype(ml_dtypes.bfloat16)),
        ))
    return plan, streams


# ---------------- kernel builders ----------------


P = 128
H = 8
POISON = -1.0e30
MAXNIDX1 = 512           # edge1 gather call size (measured faster at 512)
MAXNIDX2 = 1024          # edge2 gather call size (measured faster at 1024)
NQ = 4
USE_PT_LAYERS = (1,)     # a_d via PT-transpose matmul instead of dc-gather
F32 = mybir.dt.float32
BF16 = mybir.dt.bfloat16
I16 = mybir.dt.int16
AF = mybir.ActivationFunctionType
ALU = mybir.AluOpType
AX = mybir.AxisListType


def build_dense1(dims):
    NPC = dims.NPC
    nc = bacc.Bacc(None, target_bir_lowering=False, num_swdge_queues=NQ)
    with tile.TileContext(nc) as tc:
        with tc.tile_pool(name="dram", bufs=1, space="DRAM") as dram:
            xT = dram.tile([P, NPC], BF16, kind="ExternalInput")
            W1 = dram.tile([P, 256], F32, kind="ExternalInput")
            att1 = dram.tile([1, 512], F32, kind="ExternalInput")
            T1 = dram.tile([NPC, 384], BF16, kind="ExternalOutput")
            AD1 = dram.tile([NPC + 1, 64], F32, kind="ExternalOutput")
            names = dict(xT=xT.name, W1=W1.name, att1=att1.name,
                         T1=T1.name, AD1=AD1.name)
            with tc.tile_pool(name="cst", bufs=1) as cst, \
                 tc.tile_pool(name="wk", bufs=3) as wk, \
                 tc.tile_pool(name="ps", bufs=4, space="PSUM") as ps:
                xTs = cst.tile([P, NPC], BF16)
                nc.sync.dma_start(xTs[:], xT[:])
                rhs = cst.tile([P, 272], F32)
                nc.sync.dma_start(rhs[:, 0:256], W1[:])
                att_s = cst.tile([1, 512], F32)
                nc.sync.dma_start(att_s[:], att1[:])
                attb = cst.tile([P, 512], F32)
                nc.gpsimd.partition_broadcast(attb[:, 0:256], att_s[0:1, 0:256])
                nc.gpsimd.partition_broadcast(attb[:, 256:512], att_s[0:1, 256:512])
                tmp = cst.tile([P, 512], F32)
                nc.vector.tensor_tensor(out=tmp[:, 0:256], in0=rhs[:, 0:256],
                                        in1=attb[:, 0:256], op=ALU.mult)
                nc.vector.tensor_tensor(out=tmp[:, 256:512], in0=rhs[:, 0:256],
                                        in1=attb[:, 256:512], op=ALU.mult)
                # W1/att1 arrive column-permuted to (f*8+h); a_s[h] = sum_f
                # via the strided [h, f] view
                tv = tmp[:].rearrange("p (v f h) -> p v h f", v=2, h=H)
                nc.vector.tensor_reduce(out=rhs[:, 256:264], in_=tv[:, 0],
                                        axis=AX.X, op=ALU.add)
                nc.vector.tensor_reduce(out=rhs[:, 264:272], in_=tv[:, 1],
                                        axis=AX.X, op=ALU.add)
                rhs_bf = cst.tile([P, 272], BF16)
                nc.vector.tensor_copy(rhs_bf[:], rhs[:])
                ntile = (NPC + P - 1) // P
                for i in range(ntile):
                    r = min(P, NPC - i * P)
                    po = ps.tile([P, 272], F32, tag="po")
                    nc.tensor.matmul(out=po[:r, :], lhsT=xTs[:, i * P:i * P + r],
                                     rhs=rhs_bf[:], start=True, stop=True)
                    t1t = wk.tile([P, 384], BF16, tag="t1t")
                    nc.vector.memset(t1t[:], 0.0)
                    nc.scalar.copy(t1t[:r, 0:256], po[:r, 0:256])
                    nc.vector.tensor_copy(t1t[:, 256:384].bitcast(F32)[:r, 0:8],
                                          po[:r, 256:264])
                    nc.sync.dma_start(T1[i * P:i * P + r, :], t1t[:r, :])
                    adt = wk.tile([P, 64], F32, tag="adt")
                    nc.vector.memset(adt[:], 0.0)
                    nc.vector.tensor_copy(adt[:r, 0:8], po[:r, 264:272])
                    nc.sync.dma_start(AD1[i * P:i * P + r, :], adt[:r, :])
                pois = wk.tile([1, 64], F32, tag="pois")
                nc.vector.memset(pois[:], POISON)
                nc.sync.dma_start(AD1[NPC:NPC + 1, :], pois[:])
    nc.compile()
    return nc, names


def _split(calls, maxn):
    out = []
    scol = 0
    for (b, n, col0) in calls:
        off = 0
        while off < n:
            take = min(maxn, n - off)
            out.append((b, take, col0 + off // 16, scol))
            scol += take // P
            off += take
    return out


def build_edge(layer, plan, dims):
    N, NPC, BUCK, NB = dims.N, dims.NPC, dims.BUCK, dims.NBUCK
    windows = plan['windows']
    slots = plan['slots']
    gcols = plan['gcols']
    GW = 384 if layer == 1 else 64          # gather row width (elements)
    GDT = BF16 if layer == 1 else F32
    nc = bacc.Bacc(None, target_bir_lowering=False, num_swdge_queues=NQ)
    USE_PT = layer in USE_PT_LAYERS
    qctr = [0]

    def nextq():
        q = qctr[0] % NQ
        qctr[0] += 1
        return q

    with tile.TileContext(nc) as tc:
        with tc.tile_pool(name="dram", bufs=1, space="DRAM") as dram:
            names = {}
            Gt = dram.tile([N, GW], GDT, kind="ExternalInput")
            ADt = dram.tile([NPC + 1, 64], F32, kind="ExternalInput")
            W2 = dram.tile([32, 128], F32, kind="ExternalInput")
            nb = 32 if layer == 1 else 16
            bias = dram.tile([1, nb], F32, kind="ExternalInput")
            names.update(G=Gt.name, AD=ADt.name, W2=W2.name, bias=bias.name)
            if layer == 1:
                att2 = dram.tile([1, 256], F32, kind="ExternalInput")
                T2o = dram.tile([NPC, 64], F32, kind="ExternalOutput")
                AD2o = dram.tile([NPC + 1, 64], F32, kind="ExternalOutput")
                names.update(att2=att2.name, T2=T2o.name, AD2=AD2o.name)
            else:
                OUT = dram.tile([NPC, 16], F32, kind="ExternalOutput")
                names.update(OUT=OUT.name)
            gidx = dram.tile([P, gcols], I16, kind="ExternalInput")
            dcidx = dram.tile([P, slots * 8], I16, kind="ExternalInput")
            dstloc = dram.tile([P, slots], BF16, kind="ExternalInput")
            names.update(gidx=gidx.name, dcidx=dcidx.name, dstloc=dstloc.name)

            with tc.tile_pool(name="cst", bufs=1) as cst, \
                 tc.tile_pool(name="gp", bufs=2) as gp, \
                 tc.tile_pool(name="wk", bufs=2) as wk, \
                 tc.tile_pool(name="psa", bufs=2, space="PSUM") as psa, \
                 tc.tile_pool(name="pst", bufs=2, space="PSUM") as pst, \
                 tc.tile_pool(name="pso", bufs=2, space="PSUM") as pso, \
                 tc.tile_pool(name="psd", bufs=2, space="PSUM") as psd:
                gidx_s = cst.tile([P, gcols], I16)
                nc.sync.dma_start(gidx_s[:], gidx[:])
                dcidx_s = cst.tile([P, slots * 8], I16)
                nc.sync.dma_start(dcidx_s[:], dcidx[:])
                dstloc_s = cst.tile([P, slots], BF16)
                nc.sync.dma_start(dstloc_s[:], dstloc[:])
                iota_i = cst.tile([P, P], mybir.dt.int32)
                nc.gpsimd.iota(iota_i[:], pattern=[[1, P]], base=0,
                               channel_multiplier=0)
                iota_bf = cst.tile([P, P], BF16)
                nc.vector.tensor_copy(iota_bf[:], iota_i[:])
                bias_s = cst.tile([1, nb], F32)
                nc.sync.dma_start(bias_s[:], bias[:])
                bias_b = cst.tile([P, nb], F32)
                nc.gpsimd.partition_broadcast(bias_b[:], bias_s[0:1, :])
                W2s = cst.tile([32, 128], F32)
                nc.sync.dma_start(W2s[:], W2[:])
                ident_bf = cst.tile([P, P], BF16)
                make_identity(nc, ident_bf[:])
                if layer == 1:
                    att2_s = cst.tile([1, 256], F32)
                    nc.sync.dma_start(att2_s[:], att2[:])
                    att2b = cst.tile([32, 256], F32)
                    nc.gpsimd.partition_broadcast(att2b[:, 0:128], att2_s[0:1, 0:128])
                    nc.gpsimd.partition_broadcast(att2b[:, 128:256], att2_s[0:1, 128:256])
                    tmp2 = cst.tile([32, 256], F32)
                    nc.vector.tensor_tensor(out=tmp2[:, 0:128], in0=W2s[:],
                                            in1=att2b[:, 0:128], op=ALU.mult)
                    nc.vector.tensor_tensor(out=tmp2[:, 128:256], in0=W2s[:],
                                            in1=att2b[:, 128:256], op=ALU.mult)
                    t2v = tmp2[:].rearrange("p (v h f) -> p v h f", v=2, h=H)
                    wt2 = cst.tile([32, 16], F32)
                    nc.vector.tensor_reduce(out=wt2[:, 0:8], in_=t2v[:, 0],
                                            axis=AX.X, op=ALU.add)
                    nc.vector.tensor_reduce(out=wt2[:, 8:16], in_=t2v[:, 1],
                                            axis=AX.X, op=ALU.add)
                    ident = cst.tile([P, P], F32)
                    make_identity(nc, ident[:])
                else:
                    w2r0 = cst.tile([P, 16], BF16)
                    w2r1 = cst.tile([P, 16], BF16)
                    for h in range(H):
                        dst = w2r0 if h < 4 else w2r1
                        nc.vector.tensor_copy(
                            dst[(h % 4) * 32:(h % 4 + 1) * 32, :],
                            W2s[:, h * 16:(h + 1) * 16])

                maxn = MAXNIDX1 if layer == 1 else MAXNIDX2
                for w, win in enumerate(windows):
                    S = win['S']
                    r = min(P, NPC - w * P)
                    g_t = gp.tile([P, S, GW], GDT, tag="g")
                    for (b, n, gc0, scol) in _split(win['calls'], maxn):
                        nc.gpsimd.dma_gather(
                            g_t[:, scol:scol + n // P, :],
                            Gt[b * BUCK:(b + 1) * BUCK, :],
                            gidx_s[:, gc0:gc0 + n // 16], n, n, GW,
                            queue_num=nextq())
                    if not USE_PT:
                        dc_t = gp.tile([P, S, 64], F32, tag="dc")
                        for (_, n, dc0, scol) in _split(
                                [(0, S * P, win['slot0'] * 8)], maxn):
                            nc.gpsimd.dma_gather(
                                dc_t[:, scol:scol + n // P, :], ADt[:],
                                dcidx_s[:, dc0:dc0 + n // 16], n, n, 64,
                                queue_num=nextq())
                    if layer == 1:
                        a_s_ap = g_t[:].rearrange("p s e -> p (s e)") \
                            .bitcast(F32).rearrange("p (s e) -> p s e", e=192)[:, :, 128:136]
                        feats = g_t[:, :, 0:256]
                    else:
                        a_s_ap = g_t[:, :, 32:40]
                        gb_t = wk.tile([P, S, 32], BF16, tag="gb")
                        nc.vector.tensor_copy(gb_t[:], g_t[:, :, 0:32])
                        feats = gb_t[:]
                    p_t = wk.tile([P, S, P], BF16, tag="pt")
                    dl_b = dstloc_s[:, win['slot0']:win['slot0'] + S] \
                        .unsqueeze(2).to_broadcast([P, S, P])
                    io_b = iota_bf[:].unsqueeze(1).to_broadcast([P, S, P])
                    nc.vector.tensor_tensor(out=p_t[:], in0=dl_b, in1=io_b,
                                            op=ALU.is_equal)
                    if USE_PT:
                        adw = wk.tile([P, 8], F32, tag="adw")
                        nc.vector.memset(adw[:], 0.0)
                        nc.sync.dma_start(adw[:r, :], ADt[w * P:w * P + r, 0:8])
                        adw_b = wk.tile([P, 8], BF16, tag="adwb")
                        nc.vector.tensor_copy(adw_b[:], adw[:])
                        adE = psd.tile([P, S * 8], F32, tag="adE")
                        for s in range(S):
                            ptp = pst.tile([P, P], BF16, tag="tps")
                            nc.tensor.transpose(ptp[:], p_t[:, s, :], ident_bf[:])
                            pts = wk.tile([P, P], BF16, tag="pts")
                            nc.scalar.copy(pts[:], ptp[:])
                            nc.tensor.matmul(out=adE[:, s * 8:(s + 1) * 8],
                                             lhsT=pts[:], rhs=adw_b[:],
                                             start=True, stop=True)
                        ad_ap = adE[:].rearrange("p (s e) -> p s e", e=8)
                    else:
                        ad_ap = dc_t[:, :, 0:8]
                    et = wk.tile([P, S, 8], F32, tag="et")
                    nc.vector.tensor_tensor(out=et[:], in0=a_s_ap,
                                            in1=ad_ap, op=ALU.add)
                    nc.vector.scalar_tensor_tensor(
                        out=et[:], in0=et[:], scalar=0.2, in1=et[:],
                        op0=ALU.mult, op1=ALU.max)
                    rhs_t = wk.tile([P, S, 264], BF16, tag="rhs")
                    nc.scalar.activation(rhs_t[:, :, 256:264], et[:], AF.Exp)
                    if layer == 1:
                        # f-outer/h-inner feature layout: w broadcast sits in
                        # a middle dim, all innermost dims stride-1 bf16 ->
                        # DVE 2x_1p on the big multiply
                        wexp_b = rhs_t[:, :, 256:264].unsqueeze(2) \
                            .to_broadcast([P, S, 32, H])
                        g_v = feats.rearrange("p s (f h) -> p s f h", h=H)
                        nc.vector.tensor_tensor(
                            out=rhs_t[:, :, 0:256].rearrange(
                                "p s (f h) -> p s f h", h=H),
                            in0=g_v, in1=wexp_b, op=ALU.mult)
                    else:
                        wexp_b = rhs_t[:, :, 256:264].unsqueeze(3) \
                            .to_broadcast([P, S, 8, 32])
                        g_v = feats.unsqueeze(2).to_broadcast([P, S, 8, 32])
                        nc.vector.tensor_tensor(
                            out=rhs_t[:, :, 0:256].rearrange(
                                "p s (h f) -> p s h f", h=H),
                            in0=g_v, in1=wexp_b, op=ALU.mult)
                    agg = psa.tile([P, 264], F32, tag="agg")
                    for s in range(S):
                        nc.tensor.matmul(out=agg[:], lhsT=p_t[:, s, :],
                                         rhs=rhs_t[:, s, :],
                                         start=(s == 0), stop=(s == S - 1))
                    zr = wk.tile([P, 8], F32, tag="zr")
                    nc.vector.tensor_scalar_add(zr[:], agg[:, 256:264], 1e-16)
                    nc.vector.reciprocal(zr[:], zr[:])
                    if layer == 1:
                        zrb = zr[:].unsqueeze(1).to_broadcast([P, 32, H])
                        hn = wk.tile([P, 256], F32, tag="hn")
                        nc.vector.tensor_tensor(
                            out=hn[:].rearrange("p (f h) -> p f h", h=H),
                            in0=agg[:, 0:256].rearrange("p (f h) -> p f h", h=H),
                            in1=zrb, op=ALU.mult)
                        t2t = wk.tile([P, 64], F32, tag="t2t")
                        nc.vector.memset(t2t[:], 0.0)
                        nc.vector.tensor_reduce(
                            out=t2t[:, 0:32],
                            in_=hn[:].rearrange("p (f h) -> p f h", h=H),
                            axis=AX.X, op=ALU.add)
                        nc.scalar.mul(t2t[:, 0:32], t2t[:, 0:32], 1.0 / H)
                        nc.vector.tensor_tensor(out=t2t[:, 0:32], in0=t2t[:, 0:32],
                                                in1=bias_b[:], op=ALU.add)
                        nc.vector.tensor_scalar_max(t2t[:, 0:32], t2t[:, 0:32], 0.0)
                        hT = pso.tile([32, P], F32, tag="asd")
                        nc.tensor.transpose(hT[:], t2t[:, 0:32], ident[:])
                        hTs = wk.tile([32, P], F32, tag="hTs")
                        nc.vector.tensor_copy(hTs[:], hT[:])
                        asd = pso.tile([P, 16], F32, tag="asd")
                        nc.tensor.matmul(out=asd[:], lhsT=hTs[:], rhs=wt2[:],
                                         start=True, stop=True)
                        nc.vector.tensor_copy(t2t[:, 32:40], asd[:, 0:8])
                        nc.sync.dma_start(T2o[w * P:w * P + r, :], t2t[:r, :])
                        ad2t = wk.tile([P, 64], F32, tag="ad2t")
                        nc.vector.memset(ad2t[:], 0.0)
                        nc.vector.tensor_copy(ad2t[:, 0:8], asd[:, 8:16])
                        nc.sync.dma_start(AD2o[w * P:w * P + r, :], ad2t[:r, :])
                    else:
                        zrb = zr[:].unsqueeze(2).to_broadcast([P, H, 32])
                        anb = wk.tile([P, 256], BF16, tag="anb")
                        nc.vector.tensor_tensor(
                            out=anb[:].rearrange("p (h f) -> p h f", h=H),
                            in0=agg[:, 0:256].rearrange("p (h f) -> p h f", h=H),
                            in1=zrb, op=ALU.mult)
                        o2 = pso.tile([P, 16], F32, tag="o2")
                        for half in range(2):
                            tps = pst.tile([P, P], BF16, tag="tps")
                            nc.tensor.transpose(
                                tps[:], anb[:, half * 128:(half + 1) * 128],
                                ident_bf[:])
                            tsb = wk.tile([P, P], BF16, tag="tsb")
                            nc.vector.tensor_copy(tsb[:], tps[:])
                            nc.tensor.matmul(out=o2[:], lhsT=tsb[:],
                                             rhs=(w2r0 if half == 0 else w2r1)[:],
                                             start=(half == 0), stop=(half == 1))
                        ot = wk.tile([P, 16], F32, tag="ot")
                        nc.scalar.mul(ot[:], o2[:], 1.0 / H)
                        nc.vector.tensor_tensor(out=ot[:], in0=ot[:], in1=bias_b[:],
                                                op=ALU.add)
                        nc.sync.dma_start(OUT[w * P:w * P + r, :], ot[:r, :])
    nc.compile()
    return nc, names


# ---------------- driver ----------------


def _perm_fh():
    """column permutation: position (f*8+h) takes value from (h*32+f)"""
    perm = np.empty(256, np.int64)
    for f in range(32):
        for h in range(H):
            perm[f * H + h] = h * 32 + f
    return perm


def _run_pipeline(inputs, dims, trace=False):
    x = np.ascontiguousarray(np.asarray(inputs['x'], np.float32))
    ei = np.asarray(inputs['edge_index'])
    W1 = np.ascontiguousarray(np.asarray(inputs['W1'], np.float32))
    as1 = np.asarray(inputs['att_src1'], np.float32)
    ad1 = np.asarray(inputs['att_dst1'], np.float32)
    b1 = np.asarray(inputs['b1'], np.float32)
    W2 = np.ascontiguousarray(np.asarray(inputs['W2'], np.float32))
    as2 = np.asarray(inputs['att_src2'], np.float32)
    ad2 = np.asarray(inputs['att_dst2'], np.float32)
    b2 = np.asarray(inputs['b2'], np.float32)
    NC, NPC = dims.NCORES, dims.NPC

    plan, streams = build_uniform_plans(ei, dims)
    times = {}

    nc1, n1 = build_dense1(dims)
    perm = _perm_fh()
    W1p = np.ascontiguousarray(W1[:, perm])
    att1 = np.ascontiguousarray(np.concatenate(
        [as1.reshape(-1)[perm], ad1.reshape(-1)[perm]]
    ).reshape(1, -1).astype(np.float32))
    xTb = np.ascontiguousarray(x.T.astype(ml_dtypes.bfloat16))
    ins1 = [{n1['xT']: np.ascontiguousarray(xTb[:, c * NPC:(c + 1) * NPC]),
             n1['W1']: W1p, n1['att1']: att1} for c in range(NC)]
    r1 = bass_utils.run_bass_kernel_spmd(nc1, ins1, core_ids=list(range(NC)),
                                         trace=trace)
    times['dense1'] = r1.exec_time_ns
    T1full = np.concatenate([r1.results[c][n1['T1']] for c in range(NC)])
    ad1_shards = [r1.results[c][n1['AD1']] for c in range(NC)]

    nc2, n2 = build_edge(1, plan, dims)
    att2 = np.ascontiguousarray(np.concatenate(
        [as2.reshape(-1), ad2.reshape(-1)]).reshape(1, -1).astype(np.float32))
    ins2 = [{n2['G']: T1full, n2['AD']: ad1_shards[c], n2['W2']: W2,
             n2['att2']: att2,
             n2['bias']: np.ascontiguousarray(b1.reshape(1, -1)),
             n2['gidx']: streams[c]['gidx'], n2['dcidx']: streams[c]['dcidx'],
             n2['dstloc']: streams[c]['dstloc']} for c in range(NC)]
    r2 = bass_utils.run_bass_kernel_spmd(nc2, ins2, core_ids=list(range(NC)),
                                         trace=trace)
    times['edge1'] = r2.exec_time_ns
    T2full = np.concatenate([r2.results[c][n2['T2']] for c in range(NC)])
    ad2_shards = []
    for c in range(NC):
        a = r2.results[c][n2['AD2']].copy()
        a[NPC, :] = POISON
        ad2_shards.append(a)

    nc3, n3 = build_edge(2, plan, dims)
    ins3 = [{n3['G']: T2full, n3['AD']: ad2_shards[c], n3['W2']: W2,
             n3['bias']: np.ascontiguousarray(b2.reshape(1, -1)),
             n3['gidx']: streams[c]['gidx'], n3['dcidx']: streams[c]['dcidx'],
             n3['dstloc']: streams[c]['dstloc']} for c in range(NC)]
    r3 = bass_utils.run_bass_kernel_spmd(nc3, ins3, core_ids=list(range(NC)),
                                         trace=trace)
    times['edge2'] = r3.exec_time_ns
    out = np.concatenate([r3.results[c][n3['OUT']] for c in range(NC)])
    return np.ascontiguousarray(out.astype(np.float32)), times


def kernel(**inputs):
    out, _ = _run_pipeline(inputs, Dims(), trace=False)
    return out
